# revision 2
# baseline (speedup 1.0000x reference)
"""Trainium2 Bass kernel for the MHSA bottleneck block (v2).

Contract: kernel(**inputs) takes FULL unsharded inputs, returns FULL
[64, 2048, 14, 14] float32 output. Data-parallel over batch: 8 images/core.

v2 design:
- fp8e4m3 DoubleRow matmuls for conv1/qkv/conv3 (2x PE throughput), bf16
  attention core (logits, softmax, attn-out).
- Transposed-logits attention: logitsT[m,n] accumulated directly (ccT via
  lhsT=k, rhs=q; cpT via lhsT=q, rhs=pos) so softmax needs no transposes;
  partition-dim sums via ones-matmul, 1/s broadcast via rank-1 matmul.
- Per-image columns padded 196->208 so DoubleRow weight APs land on
  16B-aligned offsets/strides (s3_lw_dual_fp8 ISA restriction).
"""
import sys

sys.path.insert(0, '/opt/trn_rl_repo')

import numpy as np
import ml_dtypes

# Problem constants.
B, CIN, P, H, W = 64, 2048, 512, 14, 14
EPS = 1e-5
N = H * W            # 196
NCORES = 8
BPC = B // NCORES    # 8 images per core
NPAIR = BPC // 2     # 4 image pairs
KC1 = CIN // 128     # 16
KCP = KC1 // 2       # 8 input-channel pair-chunks
PC = P // 128        # 4
PCP = PC // 2        # 2
NCHUNKS = [(0, 128), (128, 68)]

IW = 208             # padded per-image column width (16B aligned)
PW = 2 * IW          # 416 per pair
TW = BPC * IW        # 1664 per core

# Per-stage dtype toggles (fp8 = e4m3 DoubleRow; else bf16).
FP8_C1 = True
FP8_QKV = True       # qk + v projections (h1 stored fp8)
FP8_C3 = True        # conv3 (h2 stored fp8)

WSC = 64.0           # weight quantization pre-scale for fp8
H2S = 8.0 if FP8_C3 else 1.0   # h2 storage pre-scale

F8 = ml_dtypes.float8_e4m3
BF16 = ml_dtypes.bfloat16

_CACHE = {}


def _build():
    import concourse.bass as bass  # noqa: F401
    import concourse.mybir as mybir
    import concourse.tile as tile
    from concourse import bacc

    f32 = mybir.dt.float32
    bf16 = mybir.dt.bfloat16
    fp8 = mybir.dt.float8e4
    DR = mybir.MatmulPerfMode.DoubleRow
    Exp = mybir.ActivationFunctionType.Exp
    Relu = mybir.ActivationFunctionType.Relu
    Copy = mybir.ActivationFunctionType.Copy
    mult = mybir.AluOpType.mult
    add = mybir.AluOpType.add

    dt_h1 = fp8 if FP8_QKV else bf16
    dt_h2 = fp8 if FP8_C3 else bf16

    nc = bacc.Bacc(None, target_bir_lowering=False, debug=False)

    # ---- DRAM parameters ----
    if FP8_C1:
        x8_d = nc.declare_dram_parameter("x8", [NPAIR, KCP, 128, 2 * PW],
                                         fp8, isOutput=False)
        w1_d = nc.declare_dram_parameter("w1q", [KCP, 128, 2 * P], fp8,
                                         isOutput=False)
    else:
        w1_d = nc.declare_dram_parameter("w1q", [KC1, 128, P], bf16,
                                         isOutput=False)
    xb_d = nc.declare_dram_parameter("xb", [KC1, 128, TW], bf16,
                                     isOutput=False)
    if FP8_QKV:
        wqk_d = nc.declare_dram_parameter("wqk", [PCP, 128, 4 * P], fp8,
                                          isOutput=False)
        wv_d = nc.declare_dram_parameter("wv", [PCP, 128, 2 * P], fp8,
                                         isOutput=False)
    else:
        wqk_d = nc.declare_dram_parameter("wqk", [PC, 128, 2 * P], bf16,
                                          isOutput=False)
        wv_d = nc.declare_dram_parameter("wv", [PC, 128, P], bf16,
                                         isOutput=False)
    if FP8_C3:
        w3_d = nc.declare_dram_parameter("w3q", [PCP, 128, 2 * CIN], fp8,
                                         isOutput=False)
    else:
        w3_d = nc.declare_dram_parameter("w3q", [PC, 128, CIN], bf16,
                                         isOutput=False)
    pos_d = nc.declare_dram_parameter("pos", [PC, 128, N], bf16,
                                      isOutput=False)
    # t1 | s2p | t2p | t3 packed as [128, 4+4+4+16]
    tsc_d = nc.declare_dram_parameter("tsc", [128, 3 * PC + KC1], f32,
                                      isOutput=False)
    y_d = nc.declare_dram_parameter("y", [KC1, 128, TW], bf16,
                                    isOutput=True)

    with tile.TileContext(nc) as tc:
        with (
            tc.tile_pool(name="const", bufs=1) as const,
            tc.tile_pool(name="xpool", bufs=1) as xpool,
            tc.tile_pool(name="h1p", bufs=2) as h1p,
            tc.tile_pool(name="qkp", bufs=2) as qkp,
            tc.tile_pool(name="vtp", bufs=4) as vtp,
            tc.tile_pool(name="atp", bufs=4) as atp,
            tc.tile_pool(name="h2p", bufs=2) as h2p,
            tc.tile_pool(name="outp", bufs=2) as outp,
            tc.tile_pool(name="psA", bufs=4, space="PSUM") as psA,
            tc.tile_pool(name="psB", bufs=3, space="PSUM") as psB,
            tc.tile_pool(name="psC", bufs=1, space="PSUM") as psC,
        ):
            # ---- weights / constants ----
            # DMA issue order matters: the sync engine issues descriptors
            # sequentially (~0.7-1.1us each) and transfers compete for HBM
            # BW, so use few large DMAs ordered by when compute needs them.
            tsc = const.tile([128, 3 * PC + KC1], f32)
            nc.sync.dma_start(out=tsc, in_=tsc_d[:, :])
            t1 = tsc[:, 0:PC]
            s2p = tsc[:, PC:2 * PC]
            t2p = tsc[:, 2 * PC:3 * PC]
            t3 = tsc[:, 3 * PC:]
            pos_t = const.tile([128, PC, N], bf16)
            nc.sync.dma_start(
                out=pos_t, in_=pos_d[:, :, :].rearrange("k p n -> p k n"))
            ones_c = const.tile([128, 128], bf16)
            nc.vector.memset(ones_c, 1.0)

            if FP8_C1:
                w1_t = const.tile([128, KCP, 2, P], fp8)
                nc.sync.dma_start(
                    out=w1_t.rearrange("p k t o -> p k (t o)"),
                    in_=w1_d[:, :, :].rearrange("k p o -> p k o"))
            else:
                w1_t = const.tile([128, KC1, P], bf16)
                nc.sync.dma_start(
                    out=w1_t, in_=w1_d[:, :, :].rearrange("k p o -> p k o"))
            # x resident for the whole core batch; DMA'd per pair slice.
            # x8 SBUF layout is pair-major so each pair is one contiguous DMA.
            if FP8_C1:
                x8_t = xpool.tile([128, NPAIR, KCP, 2, PW], fp8)
            xb_t = xpool.tile([128, KC1, TW], bf16)

            def dma_x_pair(pair):
                if FP8_C1:
                    nc.sync.dma_start(
                        out=x8_t[:, pair].rearrange("p k t n -> p k (t n)"),
                        in_=x8_d[pair].rearrange("k p o -> p k o"))
                else:
                    psl = slice(pair * PW, (pair + 1) * PW)
                    nc.sync.dma_start(
                        out=xb_t[:, :, psl],
                        in_=xb_d[:, :, psl].rearrange("k p n -> p k n"))

            def dma_xb_pair(pair):
                if not FP8_C1:
                    return  # already loaded by dma_x_pair
                psl = slice(pair * PW, (pair + 1) * PW)
                nc.sync.dma_start(
                    out=xb_t[:, :, psl],
                    in_=xb_d[:, :, psl].rearrange("k p n -> p k n"))

            dma_x_pair(0)
            if FP8_QKV:
                wqk_t = const.tile([128, PCP, 2, 2 * P], fp8)
                nc.sync.dma_start(
                    out=wqk_t.rearrange("p k t o -> p k (t o)"),
                    in_=wqk_d[:, :, :].rearrange("k p o -> p k o"))
                wv_t = const.tile([128, PCP, 2, P], fp8)
                nc.sync.dma_start(
                    out=wv_t.rearrange("p k t o -> p k (t o)"),
                    in_=wv_d[:, :, :].rearrange("k p o -> p k o"))
            else:
                wqk_t = const.tile([128, PC, 2 * P], bf16)
                nc.sync.dma_start(
                    out=wqk_t, in_=wqk_d[:, :, :].rearrange("k p o -> p k o"))
                wv_t = const.tile([128, PC, P], bf16)
                nc.sync.dma_start(
                    out=wv_t, in_=wv_d[:, :, :].rearrange("k p o -> p k o"))
            if FP8_C3:
                w3_t = const.tile([128, PCP, 2, CIN], fp8)
                nc.sync.dma_start(
                    out=w3_t.rearrange("p k t o -> p k (t o)"),
                    in_=w3_d[:, :, :].rearrange("k p o -> p k o"))
            else:
                w3_t = const.tile([128, PC, CIN], bf16)
                nc.sync.dma_start(
                    out=w3_t, in_=w3_d[:, :, :].rearrange("k p o -> p k o"))
            dma_xb_pair(0)
            dma_x_pair(1)
            for pr in range(1, NPAIR):
                dma_xb_pair(pr)
                if pr + 1 < NPAIR:
                    dma_x_pair(pr + 1)

            C1SC = 2.0 ** -6 if FP8_C1 else 1.0
            QKSC = 2.0 ** -6 if FP8_QKV else 1.0
            C3SC = (2.0 ** -9) if FP8_C3 else 1.0

            def split_j(ap_pw):
                # [128, PW] -> [128, 2, IW]
                return ap_pw.rearrange("p (j n) -> p j n", j=2)

            # PE warm-up: dummy matmuls issued before conv1(0) run while the
            # input DMAs stream, pulling the PE clock out of its low p-state
            # so real work starts at full rate.
            dmy_in = const.tile([128, 512], bf16)
            nc.vector.memset(dmy_in, 0.5)
            for i in range(40):
                wps = psB.tile([128, 512], f32, name="wps", tag="B")
                nc.tensor.matmul(wps, ones_c[:, :], dmy_in,
                                 start=True, stop=True)

            st = [dict() for _ in range(NPAIR)]  # per-pair tiles

            def conv1(pair):
                s = st[pair]
                psl = slice(pair * PW, (pair + 1) * PW)
                h1 = h1p.tile([128, PC, 2, IW], dt_h1, name=f"h1_{pair}",
                              tag="h1")
                s["h1"] = h1
                for oc in range(PC):
                    ps = psA.tile([128, 512], f32, name="c1ps", tag="A")
                    if FP8_C1:
                        for kcp in range(KCP):
                            nc.tensor.matmul(
                                ps[:, :PW],
                                w1_t[:, kcp, :, oc * 128:(oc + 1) * 128],
                                x8_t[:, pair, kcp, :, :],
                                start=(kcp == 0), stop=(kcp == KCP - 1),
                                perf_mode=DR)
                    else:
                        for kc in range(KC1):
                            nc.tensor.matmul(
                                ps[:, :PW],
                                w1_t[:, kc, oc * 128:(oc + 1) * 128],
                                xb_t[:, kc, psl],
                                start=(kc == 0), stop=(kc == KC1 - 1))
                    nc.scalar.activation(h1[:, oc, :, :],
                                         split_j(ps[:, :PW]), Relu,
                                         bias=t1[:, oc:oc + 1], scale=C1SC)

            def qk(pair):
                s = st[pair]
                h1 = s["h1"]
                q_t = qkp.tile([128, PC, 2, IW], bf16, name=f"q_{pair}",
                               tag="q")
                k_t = qkp.tile([128, PC, 2, IW], bf16, name=f"k_{pair}",
                               tag="k")
                s["q"], s["k"] = q_t, k_t
                for oc in range(2 * PC):
                    ps = psA.tile([128, 512], f32, name="qkps", tag="A")
                    if FP8_QKV:
                        for pcp in range(PCP):
                            nc.tensor.matmul(
                                ps[:, :PW],
                                wqk_t[:, pcp, :, oc * 128:(oc + 1) * 128],
                                h1[:, 2 * pcp:2 * pcp + 2, :, :],
                                start=(pcp == 0), stop=(pcp == PCP - 1),
                                perf_mode=DR)
                    else:
                        for pc in range(PC):
                            nc.tensor.matmul(
                                ps[:, :PW],
                                wqk_t[:, pc, oc * 128:(oc + 1) * 128],
                                h1[:, pc, :, :],
                                start=(pc == 0), stop=(pc == PC - 1))
                    dst = q_t if oc < PC else k_t
                    c4 = oc % PC
                    if FP8_QKV:
                        nc.vector.tensor_scalar_mul(
                            dst[:, c4, :, :], split_j(ps[:, :PW]), QKSC)
                    else:
                        nc.vector.tensor_copy(
                            dst[:, c4, :, :], split_j(ps[:, :PW]))

            def vt(pair):
                s = st[pair]
                h1 = s["h1"]
                s["vts"] = []
                for j in range(2):
                    vT = vtp.tile([128, 2, P], bf16, name=f"vT_{pair}_{j}",
                                  tag="vT")
                    for mi, (m0, msz) in enumerate(NCHUNKS):
                        ps = psA.tile([128, 512], f32, name="vps", tag="A")
                        if FP8_QKV:
                            for pcp in range(PCP):
                                nc.tensor.matmul(
                                    ps[:msz, :],
                                    h1[:, 2 * pcp:2 * pcp + 2, j,
                                       m0:m0 + msz],
                                    wv_t[:, pcp, :, :],
                                    start=(pcp == 0),
                                    stop=(pcp == PCP - 1),
                                    perf_mode=DR)
                        else:
                            for pc in range(PC):
                                nc.tensor.matmul(
                                    ps[:msz, :],
                                    h1[:, pc, j, m0:m0 + msz],
                                    wv_t[:, pc, :],
                                    start=(pc == 0), stop=(pc == PC - 1))
                        # on the scalar engine to offload DVE
                        nc.scalar.activation(vT[:msz, mi, :], ps[:msz, :],
                                             Copy, scale=QKSC)
                    s["vts"].append(vT)

            def logits(pair):
                s = st[pair]
                q_t, k_t = s["q"], s["k"]
                s["atts"] = []
                for j in range(2):
                    attnT = atp.tile([128, 2, N], bf16,
                                     name=f"aT_{pair}_{j}", tag="attnT")
                    for mi, (m0, msz) in enumerate(NCHUNKS):
                        lps = psB.tile([128, 512], f32, name="lps", tag="B")
                        for pc in range(PC):
                            nc.tensor.matmul(
                                lps[:msz, :N],
                                k_t[:, pc, j, m0:m0 + msz],
                                q_t[:, pc, j, :N],
                                start=(pc == 0), stop=False)
                        for pc in range(PC):
                            nc.tensor.matmul(
                                lps[:msz, :N],
                                q_t[:, pc, j, m0:m0 + msz],
                                pos_t[:, pc, :],
                                start=False, stop=(pc == PC - 1))
                        nc.scalar.activation(attnT[:msz, mi, :],
                                             lps[:msz, :N], Exp)
                    s["atts"].append(attnT)

            def softnorm(pair):
                # s_bc[i, n] = sum_m attnT[m, n] broadcast to all rows via
                # ones-matmul, then 1/s on all 128 DVE lanes at once.
                s = st[pair]
                s["attns"] = []
                for j in range(2):
                    attnT = s["atts"][j]
                    sps = psC.tile([128, 512], f32, name="sps", tag="C")
                    nc.tensor.matmul(sps[:, :N], ones_c[:, :],
                                     attnT[:, 0, :], start=True, stop=False)
                    nc.tensor.matmul(sps[:, :N], ones_c[:68, :],
                                     attnT[:68, 1, :], start=False,
                                     stop=True)
                    r_bc = atp.tile([128, N], f32, name=f"r_{pair}_{j}",
                                    tag="r")
                    nc.vector.reciprocal_approx_fast(r_bc, sps[:, :N])
                    attnTn = atp.tile([128, 2, N], bf16,
                                      name=f"aTn_{pair}_{j}", tag="attnTn")
                    for mi, (m0, msz) in enumerate(NCHUNKS):
                        nc.vector.tensor_tensor(
                            attnTn[:msz, mi, :], attnT[:msz, mi, :],
                            r_bc[:msz, :], mult)
                    s["attns"].append(attnTn)

            def aout(pair):
                # Both images' outputs land in one PSUM bank per c4 (one
                # accumulation group): a single activation writes h2 for
                # both images.
                s = st[pair]
                h2 = h2p.tile([128, PC, 2, IW], dt_h2, name=f"h2_{pair}",
                              tag="h2")
                s["h2"] = h2
                nc.vector.memset(h2[:, :, :, N:], 0.0)
                for c4 in range(PC):
                    aps = psB.tile([128, 512], f32, name="aps", tag="B")
                    for j in range(2):
                        vT = s["vts"][j]
                        attnTn = s["attns"][j]
                        csl = slice(j * IW, j * IW + N)
                        nc.tensor.matmul(
                            aps[:, csl],
                            vT[:128, 0, c4 * 128:(c4 + 1) * 128],
                            attnTn[:128, 0, :],
                            start=(j == 0), stop=False)
                        nc.tensor.matmul(
                            aps[:, csl],
                            vT[:68, 1, c4 * 128:(c4 + 1) * 128],
                            attnTn[:68, 1, :],
                            start=False, stop=(j == 1))
                    nc.scalar.activation(
                        h2[:, c4, :, :N], split_j(aps[:, :PW])[:, :, :N],
                        Relu, bias=t2p[:, c4:c4 + 1],
                        scale=s2p[:, c4:c4 + 1])

            def conv3y(pair):
                s = st[pair]
                h2 = s["h2"]
                psl = slice(pair * PW, (pair + 1) * PW)
                gsz = 4 if pair < NPAIR - 1 else 2
                for og in range(KC1 // gsz):
                    tmp = outp.tile([128, 4, PW], bf16, name="tmp",
                                    tag="tmp")
                    for oi in range(gsz):
                        oc = og * gsz + oi
                        ps = psA.tile([128, 512], f32, name="c3ps", tag="A")
                        if FP8_C3:
                            for pcp in range(PCP):
                                nc.tensor.matmul(
                                    ps[:, :PW],
                                    w3_t[:, pcp, :,
                                         oc * 128:(oc + 1) * 128],
                                    h2[:, 2 * pcp:2 * pcp + 2, :, :],
                                    start=(pcp == 0),
                                    stop=(pcp == PCP - 1),
                                    perf_mode=DR)
                        else:
                            for pc in range(PC):
                                nc.tensor.matmul(
                                    ps[:, :PW],
                                    w3_t[:, pc, oc * 128:(oc + 1) * 128],
                                    h2[:, pc, :, :],
                                    start=(pc == 0), stop=(pc == PC - 1))
                        # tmp = ps * C3SC + (x + t3) (t3 prefolded into xb)
                        nc.vector.scalar_tensor_tensor(
                            tmp[:, oi, :], ps[:, :PW], C3SC,
                            xb_t[:, oc, psl], op0=mult, op1=add)
                    y_sb = outp.tile([128, 4, PW], bf16, name="y_sb",
                                     tag="y_sb")
                    if FP8_C1:
                        # t3 prefolded into xb: single bias-free relu
                        nc.scalar.activation(y_sb[:, :gsz, :],
                                             tmp[:, :gsz, :], Relu)
                    else:
                        for oi in range(gsz):
                            oc = og * gsz + oi
                            nc.scalar.activation(y_sb[:, oi, :],
                                                 tmp[:, oi, :], Relu,
                                                 bias=t3[:, oc:oc + 1])
                    nc.sync.dma_start(
                        out=y_d[og * gsz:(og + 1) * gsz, :,
                                psl].rearrange("k p n -> p k n"),
                        in_=y_sb[:, :gsz, :])

            # Software pipeline: conv1(p+1) fills the exp(p) wait,
            # qk/vt/logits(p+1) fill the h2-act(p) wait, conv3y(p) overlaps
            # exp(p+1) and its y-drain overlaps softnorm/aout(p+1).
            conv1(0)
            qk(0)
            vt(0)
            logits(0)
            for p in range(NPAIR):
                if p + 1 < NPAIR:
                    conv1(p + 1)
                softnorm(p)
                aout(p)
                if p + 1 < NPAIR:
                    qk(p + 1)
                    vt(p + 1)
                    logits(p + 1)
                conv3y(p)

    nc.compile()
    return nc


def _pad_n(a):
    # pad trailing dim 196 -> 208 with zeros
    pad = [(0, 0)] * (a.ndim - 1) + [(0, IW - N)]
    return np.pad(a, pad)


def _prep_inputs(x, w1, g1, b1, m1, v1, wqkv, rel_h, rel_w,
                 g2, b2, m2, v2, w3, g3, b3, m3, v3):
    f = np.float32
    s1 = (g1 / np.sqrt(v1 + EPS)).astype(f)
    t1 = (b1 - m1 * s1).astype(f)
    s2 = (g2 / np.sqrt(v2 + EPS)).astype(f)
    t2 = (b2 - m2 * s2).astype(f)
    s3 = (g3 / np.sqrt(v3 + EPS)).astype(f)
    t3 = (b3 - m3 * s3).astype(f)

    def pack_w(wmat, fp8_flag):
        cin = wmat.shape[1]
        nout = wmat.shape[0]
        wt = np.ascontiguousarray(wmat.T.astype(f))   # [cin, out]
        if fp8_flag:
            arr = wt.reshape(cin // 256, 2, 128, nout)
            arr = np.ascontiguousarray(arr.transpose(0, 2, 1, 3))
            return np.ascontiguousarray(
                (arr * WSC).astype(F8).reshape(cin // 256, 128, 2 * nout))
        arr = wt.reshape(cin // 128, 128, nout)
        return np.ascontiguousarray(arr.astype(BF16))

    w1p = (w1 * s1[:, None]).astype(f)                 # [512, 2048]
    w1q = pack_w(w1p, FP8_C1)
    wqk = pack_w(wqkv[:2 * P].astype(f), FP8_QKV)      # [1024, 512]
    wv = pack_w(wqkv[2 * P:].astype(f), FP8_QKV)       # [512, 512]
    w3p = (w3 * s3[:, None]).astype(f)                 # [2048, 512]
    w3q = pack_w(w3p, FP8_C3)
    pos = (rel_h + rel_w).reshape(P, N).astype(f).reshape(PC, 128, N)
    pos = np.ascontiguousarray(pos.astype(BF16))

    t1_h = t1.reshape(PC, 128).T
    s2_h = (H2S * s2).reshape(PC, 128).T
    t2_h = (H2S * t2).reshape(PC, 128).T
    t3_h = t3.reshape(KC1, 128).T
    tsc = np.ascontiguousarray(
        np.concatenate([t1_h, s2_h, t2_h, t3_h], axis=1).astype(f))

    shared = dict(w1q=w1q, wqk=wqk, wv=wv, w3q=w3q, pos=pos, tsc=tsc)

    x = np.ascontiguousarray(x, f)
    in_maps = []
    for c in range(NCORES):
        xc = x[c * BPC:(c + 1) * BPC].reshape(BPC, CIN, N)
        xcp = _pad_n(xc)                                # [8, 2048, 208]
        if FP8_C1:
            # fold the conv3 bn bias into the residual (y relu is bias-free)
            xrp = xcp + t3[None, :, None]
        else:
            xrp = xcp
        # xb: [16, 128, TW] bf16
        xbc = xrp.reshape(BPC, KC1, 128, IW).transpose(1, 2, 0, 3)
        xbc = np.ascontiguousarray(xbc.reshape(KC1, 128, TW).astype(BF16))
        m = dict(shared, xb=xbc)
        if FP8_C1:
            # pair-major: [NPAIR, KCP, 128, (t, j, n)]
            x8c = xcp.reshape(NPAIR, 2, KCP, 2, 128, IW)
            x8c = x8c.transpose(0, 2, 4, 3, 1, 5)
            x8c = np.ascontiguousarray(
                x8c.reshape(NPAIR, KCP, 128, 2 * PW).astype(F8))
            m["x8"] = x8c
        in_maps.append(m)
    return in_maps


def _unpack(res):
    out = np.empty((B, CIN, H, W), np.float32)
    for c in range(NCORES):
        yc = np.asarray(res.results[c]["y"]).astype(np.float32)
        yc = yc.reshape(KC1, 128, BPC, IW)[:, :, :, :N]
        out[c * BPC:(c + 1) * BPC] = yc.transpose(2, 0, 1, 3).reshape(
            BPC, CIN, H, W)
    return out


def _run(in_maps, trace=False):
    from concourse.bass_utils import run_bass_kernel_spmd
    if "nc" not in _CACHE:
        _CACHE["nc"] = _build()
    nc = _CACHE["nc"]
    return run_bass_kernel_spmd(nc, in_maps, core_ids=list(range(NCORES)),
                                trace=trace)


def kernel(**inputs):
    in_maps = _prep_inputs(**inputs)
    res = _run(in_maps)
    return _unpack(res)
